# revision 34
# baseline (speedup 1.0000x reference)
"""DeepSeekV3-style MoE block on 8 Trainium2 NeuronCores.

Strategy (expert-parallel, host-routed dispatch/combine), fp8 DoubleRow:
  - Host computes the (tiny) sigmoid gate in fp32 numpy, does top-2 selection
    and builds per-expert token lists (the "all-to-all dispatch" happens while
    sharding the inputs).
  - Core e runs expert e's SwiGLU over its gathered tokens (padded to a fixed
    capacity) plus a 1/8 token-slice of the shared expert.  Gate scaling is
    applied on-chip.
  - The host scatter-adds the per-core outputs back into the full [B,S,H]
    tensor (the "combine" happens while unsharding).

All matmuls run as fp8e4 (e4m3) DoubleRow pairs — the PE processes two
K=128 contraction tiles per instruction at 0.5 cycles/row, 4x bf16
throughput in engine time.  bf16-level accuracy is retained with a 3-term
hi/lo decomposition per matmul:

    x @ w  ~=  x8@w8 + xl@w8 + x8@wl

where x8 = e4m3(x*Sx), xl = e4m3(x*Sx - x8) etc.  The hi/lo pair shares the
parent's power-of-2 scale so all three terms accumulate in ONE fp32 PSUM
group; scales are folded into the silu pre-scale and the host-side gate
values, so no extra descale ops run on-chip.  Per-tensor scales keep every
operand inside e4m3's normal range (weights at sigma=0.02 would otherwise
sit in the subnormal floor).

Layouts (host pre-tiled, [128, ...] partition-major, contiguous DMAs):
  phase A:  act[f,c] = silu(x@w1.T) * (x@w3.T), contraction over H
            x8/xl   [128][KH, C]   (k-tile, token) column order
            w1/w3   [128][KF, KH, 128] hi+lo
  phase B:  y[c,h]  = act.T @ w2, contraction over F (11 k-tiles x 3 terms
            = 33 products = 16 DoubleRow pairs + 1 leftover, closed with a
            duplicated w2-hi f10 slot and a zero w2 slot so ALL 17
            instructions are DoubleRow pairs with positive AP strides)
            act     [128][22, C]   slots: 11 act-hi, 11 act-lo
            w2      [128][24, H]   slots: 11 hi, 11 lo, hi-f10 dup, zeros
"""

import hashlib
import os
import sys

for _p in ("/opt/trn_rl_repo", "/opt/pypackages"):
    if _p not in sys.path:
        sys.path.append(_p)

from contextlib import ExitStack

import numpy as np
import ml_dtypes

import concourse.bacc as bacc
import concourse.mybir as mybir
import concourse.tile as tile
from concourse import bass2jax
from concourse.bass_utils import run_bass_kernel_spmd

_NEFF_CACHE_DIR = os.path.expanduser("~/.cache/bass_neff_cache")
_active_build_key = None   # set by _get_nc around the PJRT dispatch


def _install_neff_cache():
    """Persist the compiled bass_exec NEFF across processes.

    The walrus backend takes minutes for this kernel and has no cache of its
    own.  The HLO bytes are not byte-stable across processes (volatile ids /
    debug metadata), so the cache key is derived from the *build inputs*
    (capacities + CFG + build source) instead.  Only the renamed NEFF bytes
    are stored; each request re-wraps them around its own HLO."""
    if getattr(bass2jax, "_ant_neff_cache_wrapped", False):
        return
    inner = bass2jax.neuronx_cc_hook

    captured = {}
    orig_rename = bass2jax.rename_neff_tensors_and_patch_header

    def capture_rename(neff_path, mapping):
        data = orig_rename(neff_path, mapping)
        captured["neff"] = data
        return data

    bass2jax.rename_neff_tensors_and_patch_header = capture_rename

    def cached_hook(code, code_format, platform_version, file_prefix):
        c = code if isinstance(code, (bytes, bytearray)) else str(code).encode()
        if b"bass_exec" not in c or _active_build_key is None:
            return inner(code, code_format, platform_version, file_prefix)
        from libneuronxla.libncc import _wrap_neff_as_custom_call

        path = os.path.join(_NEFF_CACHE_DIR, _active_build_key + ".neff")
        try:
            if os.path.exists(path):
                with open(path, "rb") as f:
                    return 0, _wrap_neff_as_custom_call(bytes(c), f.read())
        except Exception:
            pass
        captured.pop("neff", None)
        r = inner(code, code_format, platform_version, file_prefix)
        neff = captured.pop("neff", None)
        if neff is not None:
            try:
                os.makedirs(_NEFF_CACHE_DIR, exist_ok=True)
                tmp = f"{path}.tmp{os.getpid()}"
                with open(tmp, "wb") as f:
                    f.write(neff)
                os.replace(tmp, path)
            except Exception:
                pass
        return r

    bass2jax.neuronx_cc_hook = cached_hook
    bass2jax._ant_neff_cache_wrapped = True


_install_neff_cache()


def _build_key(C_r, C_s):
    import inspect

    src = inspect.getsource(_build) + inspect.getsource(_chunks)
    blob = f"moe-ep-fp8dr-v1|{C_r}|{C_s}|{sorted(CFG.items())}|{src}"
    return hashlib.sha256(blob.encode()).hexdigest()

E4 = ml_dtypes.float8_e4m3
P = 128
H = 2048
F = 1408
E = 8
TOPK = 2
NCORES = 8
KH = H // P    # 16 contraction tiles over H
KF = F // P    # 11 contraction tiles over F
HB = H // 512  # 4 output column blocks
NW2 = 2 * KF + 2      # w2 slots: 11 hi, 11 lo, hi-f10 dup, zeros
NACT = 2 * KF         # act slots: 11 hi, 11 lo

# power-of-2 quantization scales (see module docstring)
S_X = 16.0
S_W = 512.0
S_A = 8.0
SILU_SCALE = 1.0 / (S_X * S_W)          # PSUM(A) -> h1
A32_SCALE = S_A / (S_X * S_W)           # silu(h1) * PSUM(A) -> act * S_A
GATE_SCALE = 1.0 / (S_A * S_W)          # host folds into gate values

FP32 = mybir.dt.float32
BF16_DT = mybir.dt.bfloat16
F8 = mybir.dt.float8e4


def _chunks(C, first=None):
    """Split C into 512-wide chunks (+ remainder).  `first` optionally
    shrinks the leading chunk so the kernel's first matmuls wait on a smaller
    x transfer."""
    out = []
    c0 = 0
    if first and first < C:
        out.append((0, first))
        c0 = first
    while c0 < C:
        cb = min(512, C - c0)
        out.append((c0, cb))
        c0 += cb
    return out


CFG = {
    # DMA copy count is the scarce resource: the cost model charges a fixed
    # ~600ns of the single global HWDGE device per dma_start, so every load
    # is batched into as few strided copies as possible.
    "w13_split": 1,   # dma_starts per merged w13 f-block (8KB/partition)
    "w13_split0": 2,  # f=0 split (hi half first so matmuls start sooner)
    "w13_bufs": 4,
    "x_pair0": 2,     # k-tiles in the startup-critical first x copy
    "w2_split": 6,    # dma_starts for the whole w2 slot stack
    "w2_defer_f": 2,  # emit the w2 bulk load at this f iteration
    "out_split": 1,   # dma_starts per merged output row-tile
    "ps1_bufs": 2,
    "ps2_bufs": 3,
    "o_bufs": 3,
    "silu_bufs": 3,
    "a32_bufs": 3,
    "dma_eng": "sync",  # w13 weight stream issue engine
    "x_eng": "sync",    # x load issue engine
    "w2_eng": "sync",   # bulk w2 load issue engine
    "out_eng": "gpsimd",  # output store issue engine (SWDGE: skips HWDGE,
                          # and store-waits can't head-of-line-block loads)
    "out_bf16": True,  # store outputs as bf16 (halves output DMA + tail)
    "chunk0": None,     # optional smaller leading chunk (startup latency)
    "warmup_mms": 11,   # dummy matmuls at t=0: warm the PE clock (HAM) while
                        # the first real DMAs are still in flight
}

DR = mybir.MatmulPerfMode.DoubleRow
MULT = mybir.AluOpType.mult
SUBTRACT = mybir.AluOpType.subtract


def _split_dma(eng, dst, src, n):
    w = dst.shape[-1]
    step = -(-w // n)
    for i in range(0, w, step):
        j = min(w, i + step)
        eng.dma_start(dst[:, i:j], src[:, i:j])


def _build(nc, C_r, C_s):
    """Emit the per-core program: routed expert (C_r tokens) then the shared
    expert slice (C_s tokens), both gate-scaled on-chip (the shared gate is
    the constant 1/(S_A*S_W))."""
    dram = {}
    for name, shape, dt in [
        ("xh", [P, KH * C_r], F8),
        ("xl", [P, KH * C_r], F8),
        ("gr", [P, -(-C_r // P)], FP32),
        ("w13", [P, KF * 4 * KH * P], F8),   # per-f [w1h|w3h|w1l|w3l]
        ("w2s", [P, NW2 * H], F8),
        ("sxh", [P, KH * C_s], F8),
        ("sxl", [P, KH * C_s], F8),
        ("gs", [P, -(-C_s // P)], FP32),
        ("s13", [P, KF * 4 * KH * P], F8),
        ("s2s", [P, NW2 * H], F8),
    ]:
        dram[name] = nc.dram_tensor(name, shape, dt, kind="ExternalInput")
    out_dt = BF16_DT if CFG["out_bf16"] else FP32
    yr = nc.dram_tensor("yr", [C_r, H], out_dt, kind="ExternalOutput")
    ys = nc.dram_tensor("ys", [C_s, H], out_dt, kind="ExternalOutput")

    with tile.TileContext(nc) as tc, ExitStack() as ctx:
        pool = ctx.enter_context(tc.tile_pool(name="main", bufs=1))
        psum = ctx.enter_context(tc.tile_pool(name="ps", bufs=1, space="PSUM"))
        dmae = getattr(nc, CFG["dma_eng"])
        xeng = getattr(nc, CFG["x_eng"])
        w2eng = getattr(nc, CFG["w2_eng"])
        oeng = getattr(nc, CFG["out_eng"])

        if CFG["warmup_mms"]:
            # No DMA dependency: memset SBUF (DVE — fastest to start), then
            # back-to-back matmuls so the PE HAM/p-state is warm by the time
            # the first weights arrive.
            wz = pool.tile([P, P], BF16_DT, tag="warm_w", bufs=1)
            rz = pool.tile([P, 512], BF16_DT, tag="warm_r", bufs=1)
            nc.vector.memset(wz[:], 0.0)
            nc.vector.memset(rz[:], 0.0)
            pz = psum.tile([P, 512], FP32, tag="warm_ps", bufs=1)
            for _ in range(CFG["warmup_mms"]):
                nc.tensor.matmul(pz[:], lhsT=wz[:], rhs=rz[:], start=True,
                                 stop=True)

        def problem(tag, xhd, xld, w13d, w2d, yd, C, gd, w2_defer):
            x_hi = pool.tile([P, KH * C], F8, tag=f"xh_{tag}", bufs=1)
            x_lo = pool.tile([P, KH * C], F8, tag=f"xl_{tag}", bufs=1)
            x3h = x_hi[:].rearrange("p (k c) -> p k c", k=KH)
            x3l = x_lo[:].rearrange("p (k c) -> p k c", k=KH)
            x3hd = xhd.rearrange("p (k c) -> p k c", k=KH)
            x3ld = xld.rearrange("p (k c) -> p k c", k=KH)
            g_sb = pool.tile([P, -(-C // P)], FP32, tag=f"g_{tag}", bufs=1)
            nc.sync.dma_start(g_sb[:], gd[:])

            w2_sb = pool.tile([P, NW2 * H], F8, tag="w2s", bufs=1)
            w23 = w2_sb[:].rearrange("p (s h) -> p s h", s=NW2)
            act_sb = pool.tile([P, NACT * C], F8, tag=f"act_{tag}", bufs=1)
            act3 = act_sb[:].rearrange("p (s c) -> p s c", s=NACT)

            # ---- phase A: act[f, c] = silu(x@w1.T) * (x@w3.T), [F, C] layout
            chunks = _chunks(C, first=CFG["chunk0"] if tag == "r" else None)
            # Chunk-major phase A: all KF f-tiles for chunk 0, then chunk 1,
            # ...  Only chunk-0's x is startup-critical (the later chunks'
            # loads stream during chunk 0's ~55us of matmuls), and the
            # weight stream (re-fetched per chunk, 8KB/partition per step)
            # arrives at half the rate the PE consumes it.  Costs (nchunks-1)
            # extra w13 DMA volume; the DMA device has ample slack.
            KHP = KH * P
            steps = [(c0, cb, f) for (c0, cb) in chunks for f in range(KF)]
            w13t = [pool.tile([P, 4 * KHP], F8, tag="w13",
                              bufs=CFG["w13_bufs"], name=f"w13_{tag}{i}")
                    for i in range(len(steps))]

            def load_step(i, split):
                f = steps[i][2]
                _split_dma(dmae, w13t[i][:],
                           w13d[:, f * 4 * KHP:(f + 1) * 4 * KHP], split)

            # startup-critical stream: chunk-0 x (hi leading pair first — the
            # first 8 matmuls of every group touch only hi) interleaved with
            # the first weight steps.
            cb0 = chunks[0][1]
            p0 = CFG["x_pair0"]
            load_step(0, CFG["w13_split0"])
            xeng.dma_start(x3h[:, 0:p0, 0:cb0], x3hd[:, 0:p0, 0:cb0])
            xeng.dma_start(x3h[:, p0:KH, 0:cb0], x3hd[:, p0:KH, 0:cb0])
            xeng.dma_start(x3l[:, :, 0:cb0], x3ld[:, :, 0:cb0])
            for i in range(1, min(CFG["w13_bufs"], len(steps))):
                load_step(i, CFG["w13_split"])

            pd = CFG["w13_bufs"]   # prefetch distance (in steps)
            for si, (c0, cb, f) in enumerate(steps):
                if si + pd < len(steps):
                    load_step(si + pd, CFG["w13_split"])
                # later chunks' x copies, slotted early in the step stream
                # (needed only once chunk 0's KF steps finish)
                xi = si - 1
                nchunk = 1 + xi // 2
                if 0 <= xi < 2 * (len(chunks) - 1):
                    cs, cw = chunks[nchunk]
                    xt, xd = ((x3h, x3hd) if xi % 2 == 0 else (x3l, x3ld))
                    xeng.dma_start(xt[:, :, cs:cs + cw], xd[:, :, cs:cs + cw])
                # defer the (large, phase-B-only) w2 load past startup and
                # spread it one piece per step so the burst never starves the
                # weight stream on the shared DMA device (pieces that would
                # fall past the last step are flushed there).
                n = CFG["w2_split"]
                wsz = NW2 * H
                for k in range(n):
                    due = min(w2_defer + k, len(steps) - 1)
                    if si == due:
                        lo = k * wsz // n
                        hi2 = (k + 1) * wsz // n
                        w2eng.dma_start(w2_sb[:, lo:hi2], w2d[:, lo:hi2])
                csl = slice(c0, c0 + cb)
                w13f = w13t[si]
                wt = {
                    wname: w13f[:, i * KHP:(i + 1) * KHP].rearrange(
                        "p (k j) -> p k j", k=KH)
                    for i, wname in enumerate(("w1h", "w3h", "w1l", "w3l"))
                }
                ps1 = psum.tile([P, cb], FP32, tag="ps1",
                                bufs=CFG["ps1_bufs"])
                ps3 = psum.tile([P, cb], FP32, tag="ps3",
                                bufs=CFG["ps1_bufs"])
                for ps, hi, lo in ((ps1, wt["w1h"], wt["w1l"]),
                                   (ps3, wt["w3h"], wt["w3l"])):
                    terms = [(x3h, hi), (x3l, hi), (x3h, lo)]
                    n = 0
                    for xv, wv in terms:
                        for j in range(KH // 2):
                            nc.tensor.matmul(
                                ps[:],
                                lhsT=wv[:, 2 * j:2 * j + 2, :],
                                rhs=xv[:, 2 * j:2 * j + 2, csl],
                                start=(n == 0),
                                stop=(n == 3 * (KH // 2) - 1),
                                perf_mode=DR,
                            )
                            n += 1
                tmp = pool.tile([P, cb], FP32, tag="silu",
                                bufs=CFG["silu_bufs"])
                nc.scalar.activation(
                    tmp[:], ps1[:], mybir.ActivationFunctionType.Silu,
                    scale=SILU_SCALE,
                )
                a32 = pool.tile([P, cb], FP32, tag="a32",
                                bufs=CFG["a32_bufs"])
                nc.vector.scalar_tensor_tensor(
                    a32[:], tmp[:], A32_SCALE, ps3[:], MULT, MULT)
                hi_sl = slice(f * C + c0, f * C + c0 + cb)
                lo_sl = slice((KF + f) * C + c0, (KF + f) * C + c0 + cb)
                nc.scalar.activation(
                    act_sb[:, hi_sl], a32[:],
                    mybir.ActivationFunctionType.Copy)
                nc.vector.scalar_tensor_tensor(
                    act_sb[:, lo_sl], a32[:], 1.0, act_sb[:, hi_sl],
                    MULT, SUBTRACT)

            # ---- phase B: y[c, h] = act.T @ w2, gate-scaled
            # 17 DoubleRow pairs per tile: 15 within-term + T1 f10 closed
            # with the zero dummy slots + (T3 f10, T2 f10) closed with the
            # duplicated w2-hi f10 slot.  (act slot, w2 slot) pair list:
            pair_plan = (
                [(2 * j, 1, 2 * j, 1) for j in range(KF // 2)]           # T1
                + [(KF + 2 * j, 1, 2 * j, 1) for j in range(KF // 2)]      # T2
                + [(2 * j, 1, KF + 2 * j, 1) for j in range(KF // 2)]      # T3
                + [(KF - 1, KF, KF - 1, NW2 - KF),  # T1 f10 + (actr8@zeros)
                   (KF - 1, KF, 2 * KF - 1, 1)]     # T3 f10 + T2 f10
            )
            nct = -(-C // P)
            for ct in range(nct):
                tp = min(P, C - ct * P)   # partial final token-tile
                tsl = slice(ct * P, ct * P + tp)
                # the final token-tile stores per-hb so the drain tail isn't
                # serialized behind one big transfer
                tail = ct == nct - 1
                o = pool.tile([P, HB * 512], out_dt, tag="o",
                              bufs=CFG["o_bufs"])
                for hb in range(HB):
                    hsl = slice(hb * 512, (hb + 1) * 512)
                    ps2 = psum.tile([P, 512], FP32, tag="ps2",
                                    bufs=CFG["ps2_bufs"])
                    for n, (a0, astep, w0, wstep) in enumerate(pair_plan):
                        nc.tensor.matmul(
                            ps2[:tp],
                            lhsT=act3[:, a0:a0 + astep + 1:astep, tsl],
                            rhs=w23[:, w0:w0 + wstep + 1:wstep, hsl],
                            start=(n == 0),
                            stop=(n == len(pair_plan) - 1),
                            perf_mode=DR,
                        )
                    nc.vector.tensor_scalar_mul(
                        o[:tp, hsl], ps2[:tp], g_sb[:tp, ct:ct + 1])
                    if tail:
                        # SP's queue is empty at the end; its HWDGE path has
                        # lower fixed latency than the Pool/SWDGE prep here
                        xeng.dma_start(yd[ct * P: ct * P + tp, hsl],
                                       o[:tp, hsl])
                if not tail:
                    _split_dma(
                        oeng,
                        yd[ct * P: ct * P + tp, :],
                        o[:tp, :],
                        CFG["out_split"],
                    )

        # the shared problem's w2 load overwrites the routed w2 buffer, so it
        # WAR-waits on the end of routed phase B.  Issue it AFTER all shared
        # w13 loads so that wait can't head-of-line-block the weight stream.
        problem("r", dram["xh"].ap(), dram["xl"].ap(), dram["w13"].ap(),
                dram["w2s"].ap(), yr.ap(), C_r, dram["gr"].ap(),
                CFG["w2_defer_f"])
        problem("s", dram["sxh"].ap(), dram["sxl"].ap(), dram["s13"].ap(),
                dram["s2s"].ap(), ys.ap(), C_s, dram["gs"].ap(), KF - 1)

    return nc


_cache = {}


def _get_nc(C_r, C_s):
    key = (C_r, C_s, tuple(sorted(CFG.items())))
    if key not in _cache:
        nc = bacc.Bacc("TRN2", target_bir_lowering=False, debug=False,
                       num_devices=NCORES)
        _build(nc, C_r, C_s)
        nc.compile()
        _cache[key] = nc
    return _cache[key]


def _q8(a):
    """fp32 (already scaled) -> (hi, lo) e4m3 pair, lo = residual."""
    hi = a.astype(E4)
    lo = (a - hi.astype(np.float32)).astype(E4)
    return hi, lo


def _tile_w13(w8):
    """[F, H] e4m3 -> [128, KF*KH*128], (f, kk, j) column order."""
    return np.ascontiguousarray(
        w8.reshape(KF, P, KH, P).transpose(3, 0, 2, 1)
    ).reshape(P, KF * KH * P)


def _tile_w2(w2h8, w2l8):
    """[H, F] e4m3 pair -> [128, NW2*H] slot-stacked."""
    th = np.ascontiguousarray(w2h8.reshape(H, KF, P).transpose(2, 1, 0))
    tl = np.ascontiguousarray(w2l8.reshape(H, KF, P).transpose(2, 1, 0))
    out = np.zeros((P, NW2, H), E4)
    out[:, :KF] = th
    out[:, KF:2 * KF] = tl
    out[:, 2 * KF] = th[:, KF - 1]   # duplicated hi f10 slot
    # slot 2*KF+1 stays zero (dummy-pair slot)
    return out.reshape(P, NW2 * H)


def _tile_x(x8):
    """[C, H] e4m3 -> [128, KH*C], (kk, c) column order."""
    C = x8.shape[0]
    return np.ascontiguousarray(
        x8.reshape(C, KH, P).transpose(2, 1, 0)).reshape(P, KH * C)


def _pad_rows(x, n):
    if x.shape[0] == n:
        return x
    out = np.zeros((n, x.shape[1]), x.dtype)
    out[: x.shape[0]] = x
    return out


def kernel(hidden_states, gate_w, bias, ws1, ws2, ws3, we1, we2, we3):
    orig_shape = hidden_states.shape
    x = np.ascontiguousarray(
        np.asarray(hidden_states, np.float32).reshape(-1, orig_shape[-1])
    )
    T = x.shape[0]
    gate_w = np.asarray(gate_w, np.float32)
    bias = np.asarray(bias, np.float32)
    we1 = np.asarray(we1, np.float32)
    we2 = np.asarray(we2, np.float32)
    we3 = np.asarray(we3, np.float32)
    assert gate_w.shape[0] == E and we1.shape[0] == E and x.shape[1] == H

    # ---- host router (fp32, matches the reference's selection math)
    logits = x @ gate_w.T                                 # [T, E]
    scores = np.where(
        logits >= 0,
        1.0 / (1.0 + np.exp(-np.abs(logits))),
        1.0 - 1.0 / (1.0 + np.exp(-np.abs(logits))),
    ).astype(np.float32)
    routing = scores + bias[None, :]
    topk = np.argsort(-routing, axis=1, kind="stable")[:, :TOPK]  # [T, K]
    sel = np.take_along_axis(scores, topk, axis=1)
    gates = sel / sel.sum(axis=1, keepdims=True)          # [T, K]

    idx_e = []      # token ids routed to expert e
    gate_e = []     # matching combine weights
    for e in range(E):
        mask = topk == e                      # [T, K], at most one True per row
        rows = np.nonzero(mask.any(axis=1))[0]
        idx_e.append(rows)
        gate_e.append(gates[mask].astype(np.float32))  # row-major -> rows order

    max_n = max(len(r) for r in idx_e)
    C_r = max(64, -(-max_n // 16) * 16)   # routed capacity, multiple of 16
    C_s = max(64, -(-T // (NCORES * 64)) * 64)  # shared tokens per core

    nc = _get_nc(C_r, C_s)

    # ---- quantize + tile weights (hi/lo e4m3 at power-of-2 scales)
    def pack_w13(w1, w3):
        """-> [P, KF*4*KH*P], per-f blocks [w1h | w3h | w1l | w3l]."""
        w1h, w1l = _q8(np.ascontiguousarray(w1, np.float32) * S_W)
        w3h, w3l = _q8(np.ascontiguousarray(w3, np.float32) * S_W)
        blocks = [_tile_w13(t).reshape(P, KF, KH * P)
                  for t in (w1h, w3h, w1l, w3l)]
        return np.ascontiguousarray(
            np.stack(blocks, axis=2)).reshape(P, KF * 4 * KH * P)

    s2h8, s2l8 = _q8(np.ascontiguousarray(ws2, np.float32) * S_W)
    shared_w = {
        "s13": pack_w13(ws1, ws3),
        "s2s": _tile_w2(s2h8, s2l8),
    }
    gs = np.full((-(-C_s // P) * P,), GATE_SCALE, np.float32)
    gs_t = np.ascontiguousarray(gs.reshape(-1, P).T)

    in_maps = []
    for e in range(E):
        rows = idx_e[e]
        xg = np.zeros((C_r, H), np.float32)
        xg[: len(rows)] = x[rows]
        xg *= S_X
        xh8, xl8 = _q8(xg)
        ctiles = -(-C_r // P)
        g = np.zeros((ctiles * P,), np.float32)
        g[: len(rows)] = gate_e[e] * GATE_SCALE
        w2h8, w2l8 = _q8(np.ascontiguousarray(we2[e], np.float32) * S_W)
        xs = _pad_rows(x[e * C_s: (e + 1) * C_s], C_s) * S_X
        sxh8, sxl8 = _q8(xs)
        m = {
            "xh": _tile_x(xh8),
            "xl": _tile_x(xl8),
            "gr": np.ascontiguousarray(g.reshape(ctiles, P).T),
            "w13": pack_w13(we1[e], we3[e]),
            "w2s": _tile_w2(w2h8, w2l8),
            "sxh": _tile_x(sxh8),
            "sxl": _tile_x(sxl8),
            "gs": gs_t,
        }
        m.update(shared_w)
        in_maps.append(m)

    global _active_build_key
    _active_build_key = _build_key(C_r, C_s)
    try:
        res = run_bass_kernel_spmd(nc, in_maps, list(range(NCORES))).results
    finally:
        _active_build_key = None

    # ---- host combine
    out = np.zeros((T, H), np.float32)
    for e in range(E):
        rows = idx_e[e]
        out[rows] += res[e]["yr"][: len(rows)]
        lo = e * C_s
        hi = min(T, (e + 1) * C_s)
        if lo < hi:
            out[lo:hi] += res[e]["ys"][: hi - lo]
    return out.reshape(orig_shape).astype(np.float32)


# revision 35
# speedup vs baseline: 1.0036x; 1.0036x over previous
"""DeepSeekV3-style MoE block on 8 Trainium2 NeuronCores.

Strategy (expert-parallel, host-routed dispatch/combine), fp8 DoubleRow:
  - Host computes the (tiny) sigmoid gate in fp32 numpy, does top-2 selection
    and builds per-expert token lists (the "all-to-all dispatch" happens while
    sharding the inputs).
  - Core e runs expert e's SwiGLU over its gathered tokens (padded to a fixed
    capacity) plus a 1/8 token-slice of the shared expert.  Gate scaling is
    applied on-chip.
  - The host scatter-adds the per-core outputs back into the full [B,S,H]
    tensor (the "combine" happens while unsharding).

All matmuls run as fp8e4 (e4m3) DoubleRow pairs — the PE processes two
K=128 contraction tiles per instruction at 0.5 cycles/row, 4x bf16
throughput in engine time.  bf16-level accuracy is retained with a 3-term
hi/lo decomposition per matmul:

    x @ w  ~=  x8@w8 + xl@w8 + x8@wl

where x8 = e4m3(x*Sx), xl = e4m3(x*Sx - x8) etc.  The hi/lo pair shares the
parent's power-of-2 scale so all three terms accumulate in ONE fp32 PSUM
group; scales are folded into the silu pre-scale and the host-side gate
values, so no extra descale ops run on-chip.  Per-tensor scales keep every
operand inside e4m3's normal range (weights at sigma=0.02 would otherwise
sit in the subnormal floor).

Layouts (host pre-tiled, [128, ...] partition-major, contiguous DMAs):
  phase A:  act[f,c] = silu(x@w1.T) * (x@w3.T), contraction over H
            x8/xl   [128][KH, C]   (k-tile, token) column order
            w1/w3   [128][KF, KH, 128] hi+lo
  phase B:  y[c,h]  = act.T @ w2, contraction over F (11 k-tiles x 3 terms
            = 33 products = 16 DoubleRow pairs + 1 leftover, closed with a
            duplicated w2-hi f10 slot and a zero w2 slot so ALL 17
            instructions are DoubleRow pairs with positive AP strides)
            act     [128][22, C]   slots: 11 act-hi, 11 act-lo
            w2      [128][24, H]   slots: 11 hi, 11 lo, hi-f10 dup, zeros
"""

import hashlib
import os
import sys

for _p in ("/opt/trn_rl_repo", "/opt/pypackages"):
    if _p not in sys.path:
        sys.path.append(_p)

from contextlib import ExitStack

import numpy as np
import ml_dtypes

import concourse.bacc as bacc
import concourse.mybir as mybir
import concourse.tile as tile
from concourse import bass2jax
from concourse.bass_utils import run_bass_kernel_spmd

_NEFF_CACHE_DIR = os.path.expanduser("~/.cache/bass_neff_cache")
_active_build_key = None   # set by _get_nc around the PJRT dispatch


def _install_neff_cache():
    """Persist the compiled bass_exec NEFF across processes.

    The walrus backend takes minutes for this kernel and has no cache of its
    own.  The HLO bytes are not byte-stable across processes (volatile ids /
    debug metadata), so the cache key is derived from the *build inputs*
    (capacities + CFG + build source) instead.  Only the renamed NEFF bytes
    are stored; each request re-wraps them around its own HLO."""
    if getattr(bass2jax, "_ant_neff_cache_wrapped", False):
        return
    inner = bass2jax.neuronx_cc_hook

    captured = {}
    orig_rename = bass2jax.rename_neff_tensors_and_patch_header

    def capture_rename(neff_path, mapping):
        data = orig_rename(neff_path, mapping)
        captured["neff"] = data
        return data

    bass2jax.rename_neff_tensors_and_patch_header = capture_rename

    def cached_hook(code, code_format, platform_version, file_prefix):
        c = code if isinstance(code, (bytes, bytearray)) else str(code).encode()
        if b"bass_exec" not in c or _active_build_key is None:
            return inner(code, code_format, platform_version, file_prefix)
        from libneuronxla.libncc import _wrap_neff_as_custom_call

        path = os.path.join(_NEFF_CACHE_DIR, _active_build_key + ".neff")
        try:
            if os.path.exists(path):
                with open(path, "rb") as f:
                    return 0, _wrap_neff_as_custom_call(bytes(c), f.read())
        except Exception:
            pass
        captured.pop("neff", None)
        r = inner(code, code_format, platform_version, file_prefix)
        neff = captured.pop("neff", None)
        if neff is not None:
            try:
                os.makedirs(_NEFF_CACHE_DIR, exist_ok=True)
                tmp = f"{path}.tmp{os.getpid()}"
                with open(tmp, "wb") as f:
                    f.write(neff)
                os.replace(tmp, path)
            except Exception:
                pass
        return r

    bass2jax.neuronx_cc_hook = cached_hook
    bass2jax._ant_neff_cache_wrapped = True


_install_neff_cache()


def _build_key(C_r, C_s):
    import inspect

    src = inspect.getsource(_build) + inspect.getsource(_chunks)
    blob = f"moe-ep-fp8dr-v1|{C_r}|{C_s}|{sorted(CFG.items())}|{src}"
    return hashlib.sha256(blob.encode()).hexdigest()

E4 = ml_dtypes.float8_e4m3
P = 128
H = 2048
F = 1408
E = 8
TOPK = 2
NCORES = 8
KH = H // P    # 16 contraction tiles over H
KF = F // P    # 11 contraction tiles over F
HB = H // 512  # 4 output column blocks
NW2 = 2 * KF + 2      # w2 slots: 11 hi, 11 lo, hi-f10 dup, zeros
NACT = 2 * KF         # act slots: 11 hi, 11 lo

# power-of-2 quantization scales (see module docstring)
S_X = 16.0
S_W = 512.0
S_A = 8.0
SILU_SCALE = 1.0 / (S_X * S_W)          # PSUM(A) -> h1
A32_SCALE = S_A / (S_X * S_W)           # silu(h1) * PSUM(A) -> act * S_A
GATE_SCALE = 1.0 / (S_A * S_W)          # host folds into gate values

FP32 = mybir.dt.float32
BF16_DT = mybir.dt.bfloat16
F8 = mybir.dt.float8e4


def _chunks(C, first=None):
    """Split C into 512-wide chunks (+ remainder).  `first` optionally
    shrinks the leading chunk so the kernel's first matmuls wait on a smaller
    x transfer."""
    out = []
    c0 = 0
    if first and first < C:
        out.append((0, first))
        c0 = first
    while c0 < C:
        cb = min(512, C - c0)
        out.append((c0, cb))
        c0 += cb
    return out


CFG = {
    # DMA copy count is the scarce resource: the cost model charges a fixed
    # ~600ns of the single global HWDGE device per dma_start, so every load
    # is batched into as few strided copies as possible.
    "w13_split": 1,   # dma_starts per merged w13 f-block (8KB/partition)
    "w13_split0": 2,  # f=0 split (hi half first so matmuls start sooner)
    "w13_bufs": 5,
    "x_pair0": 2,     # k-tiles in the startup-critical first x copy
    "w2_split": 6,    # dma_starts for the whole w2 slot stack
    "w2_defer_f": 2,  # emit the w2 bulk load at this f iteration
    "out_split": 1,   # dma_starts per merged output row-tile
    "ps1_bufs": 2,
    "ps2_bufs": 3,
    "o_bufs": 3,
    "silu_bufs": 3,
    "a32_bufs": 3,
    "dma_eng": "sync",  # w13 weight stream issue engine
    "x_eng": "sync",    # x load issue engine
    "w2_eng": "sync",   # bulk w2 load issue engine
    "out_eng": "gpsimd",  # output store issue engine (SWDGE: skips HWDGE,
                          # and store-waits can't head-of-line-block loads)
    "out_bf16": True,  # store outputs as bf16 (halves output DMA + tail)
    "chunk0": None,     # optional smaller leading chunk (startup latency)
    "warmup_mms": 11,   # dummy matmuls at t=0: warm the PE clock (HAM) while
                        # the first real DMAs are still in flight
}

DR = mybir.MatmulPerfMode.DoubleRow
MULT = mybir.AluOpType.mult
SUBTRACT = mybir.AluOpType.subtract


def _split_dma(eng, dst, src, n):
    w = dst.shape[-1]
    step = -(-w // n)
    for i in range(0, w, step):
        j = min(w, i + step)
        eng.dma_start(dst[:, i:j], src[:, i:j])


def _build(nc, C_r, C_s):
    """Emit the per-core program: routed expert (C_r tokens) then the shared
    expert slice (C_s tokens), both gate-scaled on-chip (the shared gate is
    the constant 1/(S_A*S_W))."""
    dram = {}
    for name, shape, dt in [
        ("xh", [P, KH * C_r], F8),
        ("xl", [P, KH * C_r], F8),
        ("gr", [P, -(-C_r // P)], FP32),
        ("w13", [P, KF * 4 * KH * P], F8),   # per-f [w1h|w3h|w1l|w3l]
        ("w2s", [P, NW2 * H], F8),
        ("sxh", [P, KH * C_s], F8),
        ("sxl", [P, KH * C_s], F8),
        ("gs", [P, -(-C_s // P)], FP32),
        ("s13", [P, KF * 4 * KH * P], F8),
        ("s2s", [P, NW2 * H], F8),
    ]:
        dram[name] = nc.dram_tensor(name, shape, dt, kind="ExternalInput")
    out_dt = BF16_DT if CFG["out_bf16"] else FP32
    yr = nc.dram_tensor("yr", [C_r, H], out_dt, kind="ExternalOutput")
    ys = nc.dram_tensor("ys", [C_s, H], out_dt, kind="ExternalOutput")

    with tile.TileContext(nc) as tc, ExitStack() as ctx:
        pool = ctx.enter_context(tc.tile_pool(name="main", bufs=1))
        psum = ctx.enter_context(tc.tile_pool(name="ps", bufs=1, space="PSUM"))
        dmae = getattr(nc, CFG["dma_eng"])
        xeng = getattr(nc, CFG["x_eng"])
        w2eng = getattr(nc, CFG["w2_eng"])
        oeng = getattr(nc, CFG["out_eng"])

        if CFG["warmup_mms"]:
            # No DMA dependency: memset SBUF (DVE — fastest to start), then
            # back-to-back matmuls so the PE HAM/p-state is warm by the time
            # the first weights arrive.
            wz = pool.tile([P, P], BF16_DT, tag="warm_w", bufs=1)
            rz = pool.tile([P, 512], BF16_DT, tag="warm_r", bufs=1)
            nc.vector.memset(wz[:], 0.0)
            nc.vector.memset(rz[:], 0.0)
            pz = psum.tile([P, 512], FP32, tag="warm_ps", bufs=1)
            for _ in range(CFG["warmup_mms"]):
                nc.tensor.matmul(pz[:], lhsT=wz[:], rhs=rz[:], start=True,
                                 stop=True)

        def problem(tag, xhd, xld, w13d, w2d, yd, C, gd, w2_defer):
            x_hi = pool.tile([P, KH * C], F8, tag=f"xh_{tag}", bufs=1)
            x_lo = pool.tile([P, KH * C], F8, tag=f"xl_{tag}", bufs=1)
            x3h = x_hi[:].rearrange("p (k c) -> p k c", k=KH)
            x3l = x_lo[:].rearrange("p (k c) -> p k c", k=KH)
            x3hd = xhd.rearrange("p (k c) -> p k c", k=KH)
            x3ld = xld.rearrange("p (k c) -> p k c", k=KH)
            g_sb = pool.tile([P, -(-C // P)], FP32, tag=f"g_{tag}", bufs=1)
            nc.sync.dma_start(g_sb[:], gd[:])

            w2_sb = pool.tile([P, NW2 * H], F8, tag="w2s", bufs=1)
            w23 = w2_sb[:].rearrange("p (s h) -> p s h", s=NW2)
            act_sb = pool.tile([P, NACT * C], F8, tag=f"act_{tag}", bufs=1)
            act3 = act_sb[:].rearrange("p (s c) -> p s c", s=NACT)

            # ---- phase A: act[f, c] = silu(x@w1.T) * (x@w3.T), [F, C] layout
            chunks = _chunks(C, first=CFG["chunk0"] if tag == "r" else None)
            # Chunk-major phase A: all KF f-tiles for chunk 0, then chunk 1,
            # ...  Only chunk-0's x is startup-critical (the later chunks'
            # loads stream during chunk 0's ~55us of matmuls), and the
            # weight stream (re-fetched per chunk, 8KB/partition per step)
            # arrives at half the rate the PE consumes it.  Costs (nchunks-1)
            # extra w13 DMA volume; the DMA device has ample slack.
            KHP = KH * P
            steps = [(c0, cb, f) for (c0, cb) in chunks for f in range(KF)]
            w13t = [pool.tile([P, 4 * KHP], F8, tag="w13",
                              bufs=CFG["w13_bufs"], name=f"w13_{tag}{i}")
                    for i in range(len(steps))]

            def load_step(i, split):
                f = steps[i][2]
                _split_dma(dmae, w13t[i][:],
                           w13d[:, f * 4 * KHP:(f + 1) * 4 * KHP], split)

            # startup-critical stream: chunk-0 x (hi leading pair first — the
            # first 8 matmuls of every group touch only hi) interleaved with
            # the first weight steps.
            cb0 = chunks[0][1]
            p0 = CFG["x_pair0"]
            load_step(0, CFG["w13_split0"])
            xeng.dma_start(x3h[:, 0:p0, 0:cb0], x3hd[:, 0:p0, 0:cb0])
            xeng.dma_start(x3h[:, p0:KH, 0:cb0], x3hd[:, p0:KH, 0:cb0])
            xeng.dma_start(x3l[:, :, 0:cb0], x3ld[:, :, 0:cb0])
            for i in range(1, min(CFG["w13_bufs"], len(steps))):
                load_step(i, CFG["w13_split"])

            pd = CFG["w13_bufs"]   # prefetch distance (in steps)
            for si, (c0, cb, f) in enumerate(steps):
                if si + pd < len(steps):
                    load_step(si + pd, CFG["w13_split"])
                # later chunks' x copies, slotted early in the step stream
                # (needed only once chunk 0's KF steps finish)
                xi = si - 1
                nchunk = 1 + xi // 2
                if 0 <= xi < 2 * (len(chunks) - 1):
                    cs, cw = chunks[nchunk]
                    xt, xd = ((x3h, x3hd) if xi % 2 == 0 else (x3l, x3ld))
                    xeng.dma_start(xt[:, :, cs:cs + cw], xd[:, :, cs:cs + cw])
                # defer the (large, phase-B-only) w2 load past startup and
                # spread it one piece per step so the burst never starves the
                # weight stream on the shared DMA device (pieces that would
                # fall past the last step are flushed there).
                n = CFG["w2_split"]
                wsz = NW2 * H
                for k in range(n):
                    due = min(w2_defer + k, len(steps) - 1)
                    if si == due:
                        lo = k * wsz // n
                        hi2 = (k + 1) * wsz // n
                        w2eng.dma_start(w2_sb[:, lo:hi2], w2d[:, lo:hi2])
                csl = slice(c0, c0 + cb)
                w13f = w13t[si]
                wt = {
                    wname: w13f[:, i * KHP:(i + 1) * KHP].rearrange(
                        "p (k j) -> p k j", k=KH)
                    for i, wname in enumerate(("w1h", "w3h", "w1l", "w3l"))
                }
                ps1 = psum.tile([P, cb], FP32, tag="ps1",
                                bufs=CFG["ps1_bufs"])
                ps3 = psum.tile([P, cb], FP32, tag="ps3",
                                bufs=CFG["ps1_bufs"])
                for ps, hi, lo in ((ps1, wt["w1h"], wt["w1l"]),
                                   (ps3, wt["w3h"], wt["w3l"])):
                    terms = [(x3h, hi), (x3l, hi), (x3h, lo)]
                    n = 0
                    for xv, wv in terms:
                        for j in range(KH // 2):
                            nc.tensor.matmul(
                                ps[:],
                                lhsT=wv[:, 2 * j:2 * j + 2, :],
                                rhs=xv[:, 2 * j:2 * j + 2, csl],
                                start=(n == 0),
                                stop=(n == 3 * (KH // 2) - 1),
                                perf_mode=DR,
                            )
                            n += 1
                tmp = pool.tile([P, cb], FP32, tag="silu",
                                bufs=CFG["silu_bufs"])
                nc.scalar.activation(
                    tmp[:], ps1[:], mybir.ActivationFunctionType.Silu,
                    scale=SILU_SCALE,
                )
                a32 = pool.tile([P, cb], FP32, tag="a32",
                                bufs=CFG["a32_bufs"])
                nc.vector.scalar_tensor_tensor(
                    a32[:], tmp[:], A32_SCALE, ps3[:], MULT, MULT)
                hi_sl = slice(f * C + c0, f * C + c0 + cb)
                lo_sl = slice((KF + f) * C + c0, (KF + f) * C + c0 + cb)
                nc.scalar.activation(
                    act_sb[:, hi_sl], a32[:],
                    mybir.ActivationFunctionType.Copy)
                nc.vector.scalar_tensor_tensor(
                    act_sb[:, lo_sl], a32[:], 1.0, act_sb[:, hi_sl],
                    MULT, SUBTRACT)

            # ---- phase B: y[c, h] = act.T @ w2, gate-scaled
            # 17 DoubleRow pairs per tile: 15 within-term + T1 f10 closed
            # with the zero dummy slots + (T3 f10, T2 f10) closed with the
            # duplicated w2-hi f10 slot.  (act slot, w2 slot) pair list:
            pair_plan = (
                [(2 * j, 1, 2 * j, 1) for j in range(KF // 2)]           # T1
                + [(KF + 2 * j, 1, 2 * j, 1) for j in range(KF // 2)]      # T2
                + [(2 * j, 1, KF + 2 * j, 1) for j in range(KF // 2)]      # T3
                + [(KF - 1, KF, KF - 1, NW2 - KF),  # T1 f10 + (actr8@zeros)
                   (KF - 1, KF, 2 * KF - 1, 1)]     # T3 f10 + T2 f10
            )
            nct = -(-C // P)
            for ct in range(nct):
                tp = min(P, C - ct * P)   # partial final token-tile
                tsl = slice(ct * P, ct * P + tp)
                # the final token-tile stores per-hb so the drain tail isn't
                # serialized behind one big transfer
                tail = ct == nct - 1
                o = pool.tile([P, HB * 512], out_dt, tag="o",
                              bufs=CFG["o_bufs"])
                for hb in range(HB):
                    hsl = slice(hb * 512, (hb + 1) * 512)
                    ps2 = psum.tile([P, 512], FP32, tag="ps2",
                                    bufs=CFG["ps2_bufs"])
                    for n, (a0, astep, w0, wstep) in enumerate(pair_plan):
                        nc.tensor.matmul(
                            ps2[:tp],
                            lhsT=act3[:, a0:a0 + astep + 1:astep, tsl],
                            rhs=w23[:, w0:w0 + wstep + 1:wstep, hsl],
                            start=(n == 0),
                            stop=(n == len(pair_plan) - 1),
                            perf_mode=DR,
                        )
                    nc.vector.tensor_scalar_mul(
                        o[:tp, hsl], ps2[:tp], g_sb[:tp, ct:ct + 1])
                    if tail:
                        # SP's queue is empty at the end; its HWDGE path has
                        # lower fixed latency than the Pool/SWDGE prep here
                        xeng.dma_start(yd[ct * P: ct * P + tp, hsl],
                                       o[:tp, hsl])
                if not tail:
                    _split_dma(
                        oeng,
                        yd[ct * P: ct * P + tp, :],
                        o[:tp, :],
                        CFG["out_split"],
                    )

        # the shared problem's w2 load overwrites the routed w2 buffer, so it
        # WAR-waits on the end of routed phase B.  Issue it AFTER all shared
        # w13 loads so that wait can't head-of-line-block the weight stream.
        problem("r", dram["xh"].ap(), dram["xl"].ap(), dram["w13"].ap(),
                dram["w2s"].ap(), yr.ap(), C_r, dram["gr"].ap(),
                CFG["w2_defer_f"])
        problem("s", dram["sxh"].ap(), dram["sxl"].ap(), dram["s13"].ap(),
                dram["s2s"].ap(), ys.ap(), C_s, dram["gs"].ap(), KF - 1)

    return nc


_cache = {}


def _get_nc(C_r, C_s):
    key = (C_r, C_s, tuple(sorted(CFG.items())))
    if key not in _cache:
        nc = bacc.Bacc("TRN2", target_bir_lowering=False, debug=False,
                       num_devices=NCORES)
        _build(nc, C_r, C_s)
        nc.compile()
        _cache[key] = nc
    return _cache[key]


def _q8(a):
    """fp32 (already scaled) -> (hi, lo) e4m3 pair, lo = residual."""
    hi = a.astype(E4)
    lo = (a - hi.astype(np.float32)).astype(E4)
    return hi, lo


def _tile_w13(w8):
    """[F, H] e4m3 -> [128, KF*KH*128], (f, kk, j) column order."""
    return np.ascontiguousarray(
        w8.reshape(KF, P, KH, P).transpose(3, 0, 2, 1)
    ).reshape(P, KF * KH * P)


def _tile_w2(w2h8, w2l8):
    """[H, F] e4m3 pair -> [128, NW2*H] slot-stacked."""
    th = np.ascontiguousarray(w2h8.reshape(H, KF, P).transpose(2, 1, 0))
    tl = np.ascontiguousarray(w2l8.reshape(H, KF, P).transpose(2, 1, 0))
    out = np.zeros((P, NW2, H), E4)
    out[:, :KF] = th
    out[:, KF:2 * KF] = tl
    out[:, 2 * KF] = th[:, KF - 1]   # duplicated hi f10 slot
    # slot 2*KF+1 stays zero (dummy-pair slot)
    return out.reshape(P, NW2 * H)


def _tile_x(x8):
    """[C, H] e4m3 -> [128, KH*C], (kk, c) column order."""
    C = x8.shape[0]
    return np.ascontiguousarray(
        x8.reshape(C, KH, P).transpose(2, 1, 0)).reshape(P, KH * C)


def _pad_rows(x, n):
    if x.shape[0] == n:
        return x
    out = np.zeros((n, x.shape[1]), x.dtype)
    out[: x.shape[0]] = x
    return out


def kernel(hidden_states, gate_w, bias, ws1, ws2, ws3, we1, we2, we3):
    orig_shape = hidden_states.shape
    x = np.ascontiguousarray(
        np.asarray(hidden_states, np.float32).reshape(-1, orig_shape[-1])
    )
    T = x.shape[0]
    gate_w = np.asarray(gate_w, np.float32)
    bias = np.asarray(bias, np.float32)
    we1 = np.asarray(we1, np.float32)
    we2 = np.asarray(we2, np.float32)
    we3 = np.asarray(we3, np.float32)
    assert gate_w.shape[0] == E and we1.shape[0] == E and x.shape[1] == H

    # ---- host router (fp32, matches the reference's selection math)
    logits = x @ gate_w.T                                 # [T, E]
    scores = np.where(
        logits >= 0,
        1.0 / (1.0 + np.exp(-np.abs(logits))),
        1.0 - 1.0 / (1.0 + np.exp(-np.abs(logits))),
    ).astype(np.float32)
    routing = scores + bias[None, :]
    topk = np.argsort(-routing, axis=1, kind="stable")[:, :TOPK]  # [T, K]
    sel = np.take_along_axis(scores, topk, axis=1)
    gates = sel / sel.sum(axis=1, keepdims=True)          # [T, K]

    idx_e = []      # token ids routed to expert e
    gate_e = []     # matching combine weights
    for e in range(E):
        mask = topk == e                      # [T, K], at most one True per row
        rows = np.nonzero(mask.any(axis=1))[0]
        idx_e.append(rows)
        gate_e.append(gates[mask].astype(np.float32))  # row-major -> rows order

    max_n = max(len(r) for r in idx_e)
    C_r = max(64, -(-max_n // 16) * 16)   # routed capacity, multiple of 16
    C_s = max(64, -(-T // (NCORES * 64)) * 64)  # shared tokens per core

    nc = _get_nc(C_r, C_s)

    # ---- quantize + tile weights (hi/lo e4m3 at power-of-2 scales)
    def pack_w13(w1, w3):
        """-> [P, KF*4*KH*P], per-f blocks [w1h | w3h | w1l | w3l]."""
        w1h, w1l = _q8(np.ascontiguousarray(w1, np.float32) * S_W)
        w3h, w3l = _q8(np.ascontiguousarray(w3, np.float32) * S_W)
        blocks = [_tile_w13(t).reshape(P, KF, KH * P)
                  for t in (w1h, w3h, w1l, w3l)]
        return np.ascontiguousarray(
            np.stack(blocks, axis=2)).reshape(P, KF * 4 * KH * P)

    s2h8, s2l8 = _q8(np.ascontiguousarray(ws2, np.float32) * S_W)
    shared_w = {
        "s13": pack_w13(ws1, ws3),
        "s2s": _tile_w2(s2h8, s2l8),
    }
    gs = np.full((-(-C_s // P) * P,), GATE_SCALE, np.float32)
    gs_t = np.ascontiguousarray(gs.reshape(-1, P).T)

    in_maps = []
    for e in range(E):
        rows = idx_e[e]
        xg = np.zeros((C_r, H), np.float32)
        xg[: len(rows)] = x[rows]
        xg *= S_X
        xh8, xl8 = _q8(xg)
        ctiles = -(-C_r // P)
        g = np.zeros((ctiles * P,), np.float32)
        g[: len(rows)] = gate_e[e] * GATE_SCALE
        w2h8, w2l8 = _q8(np.ascontiguousarray(we2[e], np.float32) * S_W)
        xs = _pad_rows(x[e * C_s: (e + 1) * C_s], C_s) * S_X
        sxh8, sxl8 = _q8(xs)
        m = {
            "xh": _tile_x(xh8),
            "xl": _tile_x(xl8),
            "gr": np.ascontiguousarray(g.reshape(ctiles, P).T),
            "w13": pack_w13(we1[e], we3[e]),
            "w2s": _tile_w2(w2h8, w2l8),
            "sxh": _tile_x(sxh8),
            "sxl": _tile_x(sxl8),
            "gs": gs_t,
        }
        m.update(shared_w)
        in_maps.append(m)

    global _active_build_key
    _active_build_key = _build_key(C_r, C_s)
    try:
        res = run_bass_kernel_spmd(nc, in_maps, list(range(NCORES))).results
    finally:
        _active_build_key = None

    # ---- host combine
    out = np.zeros((T, H), np.float32)
    for e in range(E):
        rows = idx_e[e]
        out[rows] += res[e]["yr"][: len(rows)]
        lo = e * C_s
        hi = min(T, (e + 1) * C_s)
        if lo < hi:
            out[lo:hi] += res[e]["ys"][: hi - lo]
    return out.reshape(orig_shape).astype(np.float32)
